# revision 5
# baseline (speedup 1.0000x reference)
"""DGCNN (nn_DGCNN_43911745634410) Trainium2 kernel.

Structure of the model: the only heavy compute is xw = x @ gcn1_W with
x [129, 262144] f32 (~135 MB) and gcn1_W [262144, 1] — a memory-bound matvec.
xw is shared by all three edge-attr channels (it does not depend on edge
weights). Everything downstream (segment-sums over 16K edges, a 129-element
sort, two tiny conv1ds and three FCs) is a few hundred KFLOPs.

Device strategy (8 NeuronCores, tensor-parallel over the feature dim F):
  - core c gets x[:, c*32768:(c+1)*32768] (16.5 MB) and the matching w slice;
  - a raw-Bass kernel streams the shard through SBUF and uses the DVE's fused
    scalar_tensor_tensor (out=(x*1)*w, accum_out=free-dim sum) to produce
    per-partition partial dot products at one DVE pass per element, so the
    kernel runs at the HBM/DMA roofline (~47 us per core);
  - bulk tiles are [128, 1024] (4 rows x 32 partitions-per-row), the last row
    is one short [128, 256] tile so the non-overlapped tail op is short.
  - partials ([128, 33] per core) are summed on the host in f64 (all-reduce
    across cores), and the tiny downstream runs on the host in f64, exactly
    matching the reference semantics (stable descending sort, PyG GCN
    normalization with self-loops, VALID conv1d/maxpool, ELU MLP).

The raw-Bass (no TileContext) form is deliberate: this toolchain encodes at
most ONE semaphore wait per instruction, so each x tile gets a dedicated SBUF
buffer (the whole shard fits: ~132 KB/partition of the 224 KB) and every wait
is a single explicit wait_ge.
"""
from contextlib import ExitStack

import numpy as np

import concourse.bass as bass
from concourse import mybir
from concourse.bass_utils import run_bass_kernel_spmd

F32 = mybir.dt.float32

N = 129
F = 262144
NCORES = 8
SH = F // NCORES          # 32768 features per core
FD = 1024                 # free elems per partition per bulk tile
PPR = SH // FD            # partitions per row = 32
RPT = 128 // PPR          # rows per bulk tile = 4
NFULL = ((N - 1) * SH) // (128 * FD)  # 32 bulk tiles cover rows 0..127
TFD = SH // 128           # 256: tail free-dim (row 128 as [128, 256])
NT = NFULL + 1            # 33 partial columns

_NC_CACHE = None


def _build_matvec_bass():
    nc = bass.Bass("TRN2")
    x = nc.dram_tensor("x_s", [N * SH], F32, kind="ExternalInput")
    w = nc.dram_tensor("w_rep", [128, FD], F32, kind="ExternalInput")
    wn = nc.dram_tensor("w_nat", [128, TFD], F32, kind="ExternalInput")
    out = nc.dram_tensor("part", [128, NT], F32, kind="ExternalOutput")

    with ExitStack() as ctx:
        wt = ctx.enter_context(nc.sbuf_tensor("wt", [128, FD], F32))
        wnt = ctx.enter_context(nc.sbuf_tensor("wnt", [128, TFD], F32))
        xts = [
            ctx.enter_context(nc.sbuf_tensor(f"xt{t}", [128, FD], F32))
            for t in range(NFULL)
        ]
        xtail = ctx.enter_context(nc.sbuf_tensor("xtail", [128, TFD], F32))
        part = ctx.enter_context(nc.sbuf_tensor("part_sb", [128, NT], F32))
        w_sem = ctx.enter_context(nc.semaphore("w_sem"))
        x_sems = [ctx.enter_context(nc.semaphore(f"x_sem{t}")) for t in range(NFULL)]
        tail_sem = ctx.enter_context(nc.semaphore("tail_sem"))
        dve_sem = ctx.enter_context(nc.semaphore("dve_sem"))
        out_sem = ctx.enter_context(nc.semaphore("out_sem"))
        block = ctx.enter_context(nc.Block())

        @block.sync
        def _(sync):
            sync.dma_start(wt[:, :], w[:, :]).then_inc(w_sem, 16)
            sync.dma_start(wnt[:, :], wn[:, :]).then_inc(w_sem, 16)
            for t in range(NFULL):
                src = x[t * 128 * FD : (t + 1) * 128 * FD].rearrange(
                    "(p f) -> p f", f=FD
                )
                sync.dma_start(xts[t][:, :], src).then_inc(x_sems[t], 16)
            tsrc = x[NFULL * 128 * FD :].rearrange("(p f) -> p f", f=TFD)
            sync.dma_start(xtail[:, :], tsrc).then_inc(tail_sem, 16)
            sync.wait_ge(dve_sem, NT)
            sync.dma_start(out[:, :], part[:, :]).then_inc(out_sem, 16)

        @block.vector
        def _(vector):
            vector.wait_ge(w_sem, 32)
            for t in range(NFULL):
                vector.wait_ge(x_sems[t], 16)
                nc.vector.scalar_tensor_tensor(
                    xts[t][:, :],
                    xts[t][:, :],
                    1.0,
                    wt[:, :],
                    op0=mybir.AluOpType.mult,
                    op1=mybir.AluOpType.mult,
                    accum_out=part[:, t : t + 1],
                ).then_inc(dve_sem, 1)
            vector.wait_ge(tail_sem, 16)
            nc.vector.scalar_tensor_tensor(
                xtail[:, :],
                xtail[:, :],
                1.0,
                wnt[:, :],
                op0=mybir.AluOpType.mult,
                op1=mybir.AluOpType.mult,
                accum_out=part[:, NFULL : NFULL + 1],
            ).then_inc(dve_sem, 1)

    return nc


def get_matvec_bass():
    global _NC_CACHE
    if _NC_CACHE is None:
        _NC_CACHE = _build_matvec_bass()
    return _NC_CACHE


def _make_core_inputs(x_np, w_np, core):
    xs = np.ascontiguousarray(x_np[:, core * SH : (core + 1) * SH]).reshape(-1)
    ws = w_np[core * SH : (core + 1) * SH]
    wrep = np.ascontiguousarray(np.tile(ws.reshape(PPR, FD), (RPT, 1)))
    wnat = np.ascontiguousarray(ws.reshape(128, TFD))
    return {"x_s": xs, "w_rep": wrep, "w_nat": wnat}


def _reduce_parts(parts):
    """parts: 8 arrays [128, NT] f32 -> xw [N] f64."""
    xw = np.zeros(N, np.float64)
    for part in parts:
        p = part.astype(np.float64)
        for t in range(NFULL):
            xw[RPT * t : RPT * (t + 1)] += p[:, t].reshape(RPT, PPR).sum(1)
        xw[N - 1] += p[:, NFULL].sum()
    return xw


def _matvec_device(x_np, w_np):
    """x [N, F] f32, w [F] f32 -> xw [N] f64 via the 8-core bass kernel."""
    global _NC_CACHE
    in_maps = [_make_core_inputs(x_np, w_np, c) for c in range(NCORES)]
    last_exc = None
    for attempt in range(2):
        try:
            nc = get_matvec_bass()
            res = run_bass_kernel_spmd(nc, in_maps, core_ids=list(range(NCORES)))
            return _reduce_parts([res.results[c]["part"] for c in range(NCORES)])
        except Exception as e:  # transient NRT_EXEC_UNIT_UNRECOVERABLE seen once
            import sys

            print(f"kernel: device run attempt {attempt} failed: {e!r:.200}",
                  file=sys.stderr)
            last_exc = e
            _NC_CACHE = None
    # Last-resort host fallback so a transient device failure still yields a
    # correct result (numerically equivalent partial-sum structure).
    import sys

    print(f"kernel: device path failed twice ({last_exc!r:.200}); "
          "falling back to host matvec", file=sys.stderr)
    prod = x_np.astype(np.float64) * w_np.astype(np.float64)[None, :]
    return prod.sum(axis=1)


def _downstream(xw, inputs):
    """Everything after xw = x @ gcn1_W, in f64 numpy. Returns [1, 2] f32."""
    edge_index = np.asarray(inputs["edge_index"]).astype(np.int64)
    row, col = edge_index[0], edge_index[1]
    edge_attr = np.asarray(inputs["edge_attr"], np.float64)
    g1b = np.asarray(inputs["gcn1_b"], np.float64)
    g2W = np.asarray(inputs["gcn2_W"], np.float64)
    g2b = np.asarray(inputs["gcn2_b"], np.float64)
    c1w = np.asarray(inputs["conv1_w"], np.float64)
    c1b = np.asarray(inputs["conv1_b"], np.float64)
    c2w = np.asarray(inputs["conv2_w"], np.float64)
    c2b = np.asarray(inputs["conv2_b"], np.float64)
    f1W = np.asarray(inputs["fc1_W"], np.float64)
    f1b = np.asarray(inputs["fc1_b"], np.float64)
    f2W = np.asarray(inputs["fc2_W"], np.float64)
    f2b = np.asarray(inputs["fc2_b"], np.float64)
    f3W = np.asarray(inputs["fc3_W"], np.float64)
    f3b = np.asarray(inputs["fc3_b"], np.float64)

    n = N
    loop = np.arange(n)
    row2 = np.concatenate([row, loop])
    col2 = np.concatenate([col, loop])

    def gcn(xw_vec, ew):
        # PyG GCNConv with edge weights: self-loops (weight 1), symmetric norm.
        ew2 = np.concatenate([ew, np.ones(n)])
        deg = np.zeros(n)
        np.add.at(deg, col2, ew2)
        dinv = np.where(deg > 0, deg**-0.5, 0.0)
        norm = dinv[row2] * ew2 * dinv[col2]
        out = np.zeros(n)
        np.add.at(out, col2, norm * xw_vec[row2])
        return out

    outs = []
    for c in range(3):
        ew = edge_attr[:, c]
        h1 = gcn(xw, ew) + g1b[0]
        h2 = gcn(h1 * g2W[0, 0], ew) + g2b[0]
        # SortPool: jnp.argsort(-h2) is a stable ascending sort of the negation
        perm = np.argsort(-h2, kind="stable")
        hs = np.stack([h1[perm], h2[perm]], axis=1)  # [n, 2]
        z = hs.T  # [2, n]
        L = z.shape[1] - 2
        z1 = np.zeros((3, L))
        for o in range(3):
            for i in range(2):
                for k in range(3):
                    z1[o] += c1w[o, i, k] * z[i, k : k + L]
            z1[o] += c1b[o]
        z1p = np.max(np.stack([z1[:, 0 : L - 2], z1[:, 1 : L - 1], z1[:, 2:L]], 0), 0)
        L2 = z1p.shape[1] - 2
        z2 = np.zeros((1, L2))
        for i in range(3):
            for k in range(3):
                z2[0] += c2w[0, i, k] * z1p[i, k : k + L2]
        z2[0] += c2b[0]
        z2p = np.max(
            np.stack([z2[:, 0 : L2 - 2], z2[:, 1 : L2 - 1], z2[:, 2:L2]], 0), 0
        )
        outs.append(z2p)  # [1, 121]

    allx = np.concatenate(outs, axis=0)  # [3, 121]
    h = allx.reshape(1, -1)

    def elu(v):
        return np.where(v > 0, v, np.expm1(v))

    h = elu(h @ f1W + f1b)
    h = elu(h @ f2W + f2b)
    out = h @ f3W + f3b
    return out.astype(np.float32)


def kernel(**inputs) -> np.ndarray:
    x = np.ascontiguousarray(np.asarray(inputs["x"], np.float32))
    w = np.asarray(inputs["gcn1_W"], np.float32).reshape(-1)
    xw = _matvec_device(x, w)
    return _downstream(xw, inputs)


# revision 6
# speedup vs baseline: 1.0025x; 1.0025x over previous
"""DGCNN (nn_DGCNN_43911745634410) Trainium2 kernel.

Structure of the model: the only heavy compute is xw = x @ gcn1_W with
x [129, 262144] f32 (~135 MB) and gcn1_W [262144, 1] — a memory-bound matvec.
xw is shared by all three edge-attr channels (it does not depend on edge
weights). Everything downstream (segment-sums over 16K edges, a 129-element
sort, two tiny conv1ds and three FCs) is a few hundred KFLOPs.

Device strategy (8 NeuronCores, tensor-parallel over the feature dim F):
  - core c gets x[:, c*32768:(c+1)*32768] (16.5 MB) and the matching w slice;
  - a raw-Bass kernel streams the shard through SBUF and uses the DVE's fused
    scalar_tensor_tensor (out=(x*1)*w, accum_out=free-dim sum) to produce
    per-partition partial dot products at one DVE pass per element, so the
    kernel runs at the HBM/DMA roofline (~47 us per core);
  - bulk tiles are [128, 1024] (4 rows x 32 partitions-per-row), the last row
    is one short [128, 256] tile so the non-overlapped tail op is short.
  - partials ([128, 33] per core) are summed on the host in f64 (all-reduce
    across cores), and the tiny downstream runs on the host in f64, exactly
    matching the reference semantics (stable descending sort, PyG GCN
    normalization with self-loops, VALID conv1d/maxpool, ELU MLP).

The raw-Bass (no TileContext) form is deliberate: this toolchain encodes at
most ONE semaphore wait per instruction, so each x tile gets a dedicated SBUF
buffer (the whole shard fits: ~132 KB/partition of the 224 KB) and every wait
is a single explicit wait_ge.
"""
from contextlib import ExitStack

import numpy as np

import concourse.bass as bass
from concourse import mybir
from concourse.bass_utils import run_bass_kernel_spmd

F32 = mybir.dt.float32

N = 129
F = 262144
NCORES = 8
SH = F // NCORES          # 32768 features per core
FD = 1024                 # free elems per partition per bulk tile
PPR = SH // FD            # partitions per row = 32
RPT = 128 // PPR          # rows per bulk tile = 4
NB = 31                   # bulk tiles [128, 1024], rows 0..123
TFD = SH // 128           # 256: small-tile free dim (one row per tile)
NS = 5                    # small tiles [128, 256], rows 124..128
NCOL = NB + NS            # 36 partial columns

_NC_CACHE = None


def _build_matvec_bass():
    nc = bass.Bass("TRN2")
    x = nc.dram_tensor("x_s", [N * SH], F32, kind="ExternalInput")
    w = nc.dram_tensor("w_rep", [128, FD], F32, kind="ExternalInput")
    wn = nc.dram_tensor("w_nat", [128, TFD], F32, kind="ExternalInput")
    out = nc.dram_tensor("part", [128, NCOL], F32, kind="ExternalOutput")

    with ExitStack() as ctx:
        wt = ctx.enter_context(nc.sbuf_tensor("wt", [128, FD], F32))
        wnt = ctx.enter_context(nc.sbuf_tensor("wnt", [128, TFD], F32))
        xts = [
            ctx.enter_context(nc.sbuf_tensor(f"xt{t}", [128, FD], F32))
            for t in range(NB)
        ]
        xss = [
            ctx.enter_context(nc.sbuf_tensor(f"xs{s}", [128, TFD], F32))
            for s in range(NS)
        ]
        part = ctx.enter_context(nc.sbuf_tensor("part_sb", [128, NCOL], F32))
        w_sem = ctx.enter_context(nc.semaphore("w_sem"))
        x_sems = [ctx.enter_context(nc.semaphore(f"x_sem{t}")) for t in range(NB)]
        s_sems = [ctx.enter_context(nc.semaphore(f"s_sem{s}")) for s in range(NS)]
        dve_sem = ctx.enter_context(nc.semaphore("dve_sem"))
        out_sem = ctx.enter_context(nc.semaphore("out_sem"))
        block = ctx.enter_context(nc.Block())

        base = NB * 128 * FD

        @block.sync
        def _(sync):
            sync.dma_start(wt[:, :], w[:, :]).then_inc(w_sem, 16)
            sync.dma_start(wnt[:, :], wn[:, :]).then_inc(w_sem, 16)
            for t in range(NB):
                src = x[t * 128 * FD : (t + 1) * 128 * FD].rearrange(
                    "(p f) -> p f", f=FD
                )
                sync.dma_start(xts[t][:, :], src).then_inc(x_sems[t], 16)
            # rows 124..128 as five short tiles so the closing DVE ops are short
            for si in range(NS):
                src = x[base + si * 128 * TFD : base + (si + 1) * 128 * TFD].rearrange(
                    "(p f) -> p f", f=TFD
                )
                sync.dma_start(xss[si][:, :], src).then_inc(s_sems[si], 16)
            sync.wait_ge(dve_sem, NCOL)
            sync.dma_start(out[:, :], part[:, :]).then_inc(out_sem, 16)

        @block.vector
        def _(vector):
            vector.wait_ge(w_sem, 32)
            for t in range(NB):
                vector.wait_ge(x_sems[t], 16)
                nc.vector.scalar_tensor_tensor(
                    xts[t][:, :],
                    xts[t][:, :],
                    1.0,
                    wt[:, :],
                    op0=mybir.AluOpType.mult,
                    op1=mybir.AluOpType.mult,
                    accum_out=part[:, t : t + 1],
                ).then_inc(dve_sem, 1)
            for si in range(NS):
                vector.wait_ge(s_sems[si], 16)
                nc.vector.scalar_tensor_tensor(
                    xss[si][:, :],
                    xss[si][:, :],
                    1.0,
                    wnt[:, :],
                    op0=mybir.AluOpType.mult,
                    op1=mybir.AluOpType.mult,
                    accum_out=part[:, NB + si : NB + si + 1],
                ).then_inc(dve_sem, 1)

    return nc


def get_matvec_bass():
    global _NC_CACHE
    if _NC_CACHE is None:
        _NC_CACHE = _build_matvec_bass()
    return _NC_CACHE


def _make_core_inputs(x_np, w_np, core):
    xs = np.ascontiguousarray(x_np[:, core * SH : (core + 1) * SH]).reshape(-1)
    ws = w_np[core * SH : (core + 1) * SH]
    wrep = np.ascontiguousarray(np.tile(ws.reshape(PPR, FD), (RPT, 1)))
    wnat = np.ascontiguousarray(ws.reshape(128, TFD))
    return {"x_s": xs, "w_rep": wrep, "w_nat": wnat}


def _reduce_parts(parts):
    """parts: 8 arrays [128, NCOL] f32 -> xw [N] f64."""
    xw = np.zeros(N, np.float64)
    for part in parts:
        p = part.astype(np.float64)
        for t in range(NB):
            xw[RPT * t : RPT * (t + 1)] += p[:, t].reshape(RPT, PPR).sum(1)
        for si in range(NS):
            xw[RPT * NB + si] += p[:, NB + si].sum()
    return xw


def _matvec_device(x_np, w_np):
    """x [N, F] f32, w [F] f32 -> xw [N] f64 via the 8-core bass kernel."""
    global _NC_CACHE
    in_maps = [_make_core_inputs(x_np, w_np, c) for c in range(NCORES)]
    last_exc = None
    for attempt in range(2):
        try:
            nc = get_matvec_bass()
            res = run_bass_kernel_spmd(nc, in_maps, core_ids=list(range(NCORES)))
            return _reduce_parts([res.results[c]["part"] for c in range(NCORES)])
        except Exception as e:  # transient NRT_EXEC_UNIT_UNRECOVERABLE seen once
            import sys

            print(f"kernel: device run attempt {attempt} failed: {e!r:.200}",
                  file=sys.stderr)
            last_exc = e
            _NC_CACHE = None
    # Last-resort host fallback so a transient device failure still yields a
    # correct result (numerically equivalent partial-sum structure).
    import sys

    print(f"kernel: device path failed twice ({last_exc!r:.200}); "
          "falling back to host matvec", file=sys.stderr)
    prod = x_np.astype(np.float64) * w_np.astype(np.float64)[None, :]
    return prod.sum(axis=1)


def _downstream(xw, inputs):
    """Everything after xw = x @ gcn1_W, in f64 numpy. Returns [1, 2] f32."""
    edge_index = np.asarray(inputs["edge_index"]).astype(np.int64)
    row, col = edge_index[0], edge_index[1]
    edge_attr = np.asarray(inputs["edge_attr"], np.float64)
    g1b = np.asarray(inputs["gcn1_b"], np.float64)
    g2W = np.asarray(inputs["gcn2_W"], np.float64)
    g2b = np.asarray(inputs["gcn2_b"], np.float64)
    c1w = np.asarray(inputs["conv1_w"], np.float64)
    c1b = np.asarray(inputs["conv1_b"], np.float64)
    c2w = np.asarray(inputs["conv2_w"], np.float64)
    c2b = np.asarray(inputs["conv2_b"], np.float64)
    f1W = np.asarray(inputs["fc1_W"], np.float64)
    f1b = np.asarray(inputs["fc1_b"], np.float64)
    f2W = np.asarray(inputs["fc2_W"], np.float64)
    f2b = np.asarray(inputs["fc2_b"], np.float64)
    f3W = np.asarray(inputs["fc3_W"], np.float64)
    f3b = np.asarray(inputs["fc3_b"], np.float64)

    n = N
    loop = np.arange(n)
    row2 = np.concatenate([row, loop])
    col2 = np.concatenate([col, loop])

    def gcn(xw_vec, ew):
        # PyG GCNConv with edge weights: self-loops (weight 1), symmetric norm.
        ew2 = np.concatenate([ew, np.ones(n)])
        deg = np.zeros(n)
        np.add.at(deg, col2, ew2)
        dinv = np.where(deg > 0, deg**-0.5, 0.0)
        norm = dinv[row2] * ew2 * dinv[col2]
        out = np.zeros(n)
        np.add.at(out, col2, norm * xw_vec[row2])
        return out

    outs = []
    for c in range(3):
        ew = edge_attr[:, c]
        h1 = gcn(xw, ew) + g1b[0]
        h2 = gcn(h1 * g2W[0, 0], ew) + g2b[0]
        # SortPool: jnp.argsort(-h2) is a stable ascending sort of the negation
        perm = np.argsort(-h2, kind="stable")
        hs = np.stack([h1[perm], h2[perm]], axis=1)  # [n, 2]
        z = hs.T  # [2, n]
        L = z.shape[1] - 2
        z1 = np.zeros((3, L))
        for o in range(3):
            for i in range(2):
                for k in range(3):
                    z1[o] += c1w[o, i, k] * z[i, k : k + L]
            z1[o] += c1b[o]
        z1p = np.max(np.stack([z1[:, 0 : L - 2], z1[:, 1 : L - 1], z1[:, 2:L]], 0), 0)
        L2 = z1p.shape[1] - 2
        z2 = np.zeros((1, L2))
        for i in range(3):
            for k in range(3):
                z2[0] += c2w[0, i, k] * z1p[i, k : k + L2]
        z2[0] += c2b[0]
        z2p = np.max(
            np.stack([z2[:, 0 : L2 - 2], z2[:, 1 : L2 - 1], z2[:, 2:L2]], 0), 0
        )
        outs.append(z2p)  # [1, 121]

    allx = np.concatenate(outs, axis=0)  # [3, 121]
    h = allx.reshape(1, -1)

    def elu(v):
        return np.where(v > 0, v, np.expm1(v))

    h = elu(h @ f1W + f1b)
    h = elu(h @ f2W + f2b)
    out = h @ f3W + f3b
    return out.astype(np.float32)


def kernel(**inputs) -> np.ndarray:
    x = np.ascontiguousarray(np.asarray(inputs["x"], np.float32))
    w = np.asarray(inputs["gcn1_W"], np.float32).reshape(-1)
    xw = _matvec_device(x, w)
    return _downstream(xw, inputs)


# revision 7
# speedup vs baseline: 1.0127x; 1.0101x over previous
"""DGCNN (nn_DGCNN_43911745634410) Trainium2 kernel.

Structure of the model: the only heavy compute is xw = x @ gcn1_W with
x [129, 262144] f32 (~135 MB) and gcn1_W [262144, 1] — a memory-bound matvec.
xw is shared by all three edge-attr channels (it does not depend on edge
weights). Everything downstream (segment-sums over 16K edges, a 129-element
sort, two tiny conv1ds and three FCs) is a few hundred KFLOPs.

Device strategy (8 NeuronCores, tensor-parallel over the feature dim F):
  - core c gets x[:, c*32768:(c+1)*32768] (16.5 MB) and the matching w slice;
  - a raw-Bass kernel streams the shard through SBUF and uses the DVE's fused
    scalar_tensor_tensor (out=(x*1)*w, accum_out=free-dim sum) to produce
    per-partition partial dot products at one DVE pass per element, so the
    kernel runs at the HBM/DMA roofline (~47 us per core);
  - bulk tiles are [128, 1024] (4 rows x 32 partitions-per-row), the last row
    is one short [128, 256] tile so the non-overlapped tail op is short.
  - partials ([128, 33] per core) are summed on the host in f64 (all-reduce
    across cores), and the tiny downstream runs on the host in f64, exactly
    matching the reference semantics (stable descending sort, PyG GCN
    normalization with self-loops, VALID conv1d/maxpool, ELU MLP).

The raw-Bass (no TileContext) form is deliberate: this toolchain encodes at
most ONE semaphore wait per instruction, so each x tile gets a dedicated SBUF
buffer (the whole shard fits: ~132 KB/partition of the 224 KB) and every wait
is a single explicit wait_ge.
"""
from contextlib import ExitStack

import numpy as np

import concourse.bass as bass
from concourse import mybir
from concourse.bass_utils import run_bass_kernel_spmd

F32 = mybir.dt.float32

N = 129
F = 262144
NCORES = 8
SH = F // NCORES          # 32768 features per core
FD = 1024                 # free elems per partition per bulk tile
PPR = SH // FD            # partitions per row = 32
RPT = 128 // PPR          # rows per bulk tile = 4
NB = 27                   # bulk tiles [128, 1024], rows 0..107
TFD = SH // 128           # 256: small-tile free dim (one row per tile)
NS = 21                   # small tiles [128, 256], rows 108..128
NCOL = NB + NS            # 48 partial columns

_NC_CACHE = None


def _build_matvec_bass():
    nc = bass.Bass("TRN2")
    x = nc.dram_tensor("x_s", [N * SH], F32, kind="ExternalInput")
    w = nc.dram_tensor("w_rep", [128, FD], F32, kind="ExternalInput")
    wn = nc.dram_tensor("w_nat", [128, TFD], F32, kind="ExternalInput")
    out = nc.dram_tensor("part", [128, NCOL], F32, kind="ExternalOutput")

    with ExitStack() as ctx:
        wt = ctx.enter_context(nc.sbuf_tensor("wt", [128, FD], F32))
        wnt = ctx.enter_context(nc.sbuf_tensor("wnt", [128, TFD], F32))
        xts = [
            ctx.enter_context(nc.sbuf_tensor(f"xt{t}", [128, FD], F32))
            for t in range(NB)
        ]
        xss = [
            ctx.enter_context(nc.sbuf_tensor(f"xs{s}", [128, TFD], F32))
            for s in range(NS)
        ]
        part = ctx.enter_context(nc.sbuf_tensor("part_sb", [128, NCOL], F32))
        w_sem = ctx.enter_context(nc.semaphore("w_sem"))
        x_sems = [ctx.enter_context(nc.semaphore(f"x_sem{t}")) for t in range(NB)]
        s_sems = [ctx.enter_context(nc.semaphore(f"s_sem{s}")) for s in range(NS)]
        dve_sem = ctx.enter_context(nc.semaphore("dve_sem"))
        out_sem = ctx.enter_context(nc.semaphore("out_sem"))
        block = ctx.enter_context(nc.Block())

        base = NB * 128 * FD

        @block.sync
        def _(sync):
            sync.dma_start(wt[:, :], w[:, :]).then_inc(w_sem, 16)
            sync.dma_start(wnt[:, :], wn[:, :]).then_inc(w_sem, 16)
            for t in range(NB):
                src = x[t * 128 * FD : (t + 1) * 128 * FD].rearrange(
                    "(p f) -> p f", f=FD
                )
                sync.dma_start(xts[t][:, :], src).then_inc(x_sems[t], 16)
            # closing rows as one-row short tiles: each STT starts a fixed ~1us
            # (DMA receipt+sem latency) after its tile lands, so short closing ops
            # let the DVE drain in lock-step with the last arrivals
            for si in range(NS):
                src = x[base + si * 128 * TFD : base + (si + 1) * 128 * TFD].rearrange(
                    "(p f) -> p f", f=TFD
                )
                sync.dma_start(xss[si][:, :], src).then_inc(s_sems[si], 16)
            sync.wait_ge(dve_sem, NCOL)
            sync.dma_start(out[:, :], part[:, :]).then_inc(out_sem, 16)

        @block.vector
        def _(vector):
            vector.wait_ge(w_sem, 32)
            for t in range(NB):
                vector.wait_ge(x_sems[t], 16)
                nc.vector.scalar_tensor_tensor(
                    xts[t][:, :],
                    xts[t][:, :],
                    1.0,
                    wt[:, :],
                    op0=mybir.AluOpType.mult,
                    op1=mybir.AluOpType.mult,
                    accum_out=part[:, t : t + 1],
                ).then_inc(dve_sem, 1)
            for si in range(NS):
                vector.wait_ge(s_sems[si], 16)
                nc.vector.scalar_tensor_tensor(
                    xss[si][:, :],
                    xss[si][:, :],
                    1.0,
                    wnt[:, :],
                    op0=mybir.AluOpType.mult,
                    op1=mybir.AluOpType.mult,
                    accum_out=part[:, NB + si : NB + si + 1],
                ).then_inc(dve_sem, 1)

    return nc


def get_matvec_bass():
    global _NC_CACHE
    if _NC_CACHE is None:
        _NC_CACHE = _build_matvec_bass()
    return _NC_CACHE


def _make_core_inputs(x_np, w_np, core):
    xs = np.ascontiguousarray(x_np[:, core * SH : (core + 1) * SH]).reshape(-1)
    ws = w_np[core * SH : (core + 1) * SH]
    wrep = np.ascontiguousarray(np.tile(ws.reshape(PPR, FD), (RPT, 1)))
    wnat = np.ascontiguousarray(ws.reshape(128, TFD))
    return {"x_s": xs, "w_rep": wrep, "w_nat": wnat}


def _reduce_parts(parts):
    """parts: 8 arrays [128, NCOL] f32 -> xw [N] f64."""
    xw = np.zeros(N, np.float64)
    for part in parts:
        p = part.astype(np.float64)
        for t in range(NB):
            xw[RPT * t : RPT * (t + 1)] += p[:, t].reshape(RPT, PPR).sum(1)
        for si in range(NS):
            xw[RPT * NB + si] += p[:, NB + si].sum()
    return xw


def _matvec_device(x_np, w_np):
    """x [N, F] f32, w [F] f32 -> xw [N] f64 via the 8-core bass kernel."""
    global _NC_CACHE
    in_maps = [_make_core_inputs(x_np, w_np, c) for c in range(NCORES)]
    last_exc = None
    for attempt in range(2):
        try:
            nc = get_matvec_bass()
            res = run_bass_kernel_spmd(nc, in_maps, core_ids=list(range(NCORES)))
            return _reduce_parts([res.results[c]["part"] for c in range(NCORES)])
        except Exception as e:  # transient NRT_EXEC_UNIT_UNRECOVERABLE seen once
            import sys

            print(f"kernel: device run attempt {attempt} failed: {e!r:.200}",
                  file=sys.stderr)
            last_exc = e
            _NC_CACHE = None
    # Last-resort host fallback so a transient device failure still yields a
    # correct result (numerically equivalent partial-sum structure).
    import sys

    print(f"kernel: device path failed twice ({last_exc!r:.200}); "
          "falling back to host matvec", file=sys.stderr)
    prod = x_np.astype(np.float64) * w_np.astype(np.float64)[None, :]
    return prod.sum(axis=1)


def _downstream(xw, inputs):
    """Everything after xw = x @ gcn1_W, in f64 numpy. Returns [1, 2] f32."""
    edge_index = np.asarray(inputs["edge_index"]).astype(np.int64)
    row, col = edge_index[0], edge_index[1]
    edge_attr = np.asarray(inputs["edge_attr"], np.float64)
    g1b = np.asarray(inputs["gcn1_b"], np.float64)
    g2W = np.asarray(inputs["gcn2_W"], np.float64)
    g2b = np.asarray(inputs["gcn2_b"], np.float64)
    c1w = np.asarray(inputs["conv1_w"], np.float64)
    c1b = np.asarray(inputs["conv1_b"], np.float64)
    c2w = np.asarray(inputs["conv2_w"], np.float64)
    c2b = np.asarray(inputs["conv2_b"], np.float64)
    f1W = np.asarray(inputs["fc1_W"], np.float64)
    f1b = np.asarray(inputs["fc1_b"], np.float64)
    f2W = np.asarray(inputs["fc2_W"], np.float64)
    f2b = np.asarray(inputs["fc2_b"], np.float64)
    f3W = np.asarray(inputs["fc3_W"], np.float64)
    f3b = np.asarray(inputs["fc3_b"], np.float64)

    n = N
    loop = np.arange(n)
    row2 = np.concatenate([row, loop])
    col2 = np.concatenate([col, loop])

    def gcn(xw_vec, ew):
        # PyG GCNConv with edge weights: self-loops (weight 1), symmetric norm.
        ew2 = np.concatenate([ew, np.ones(n)])
        deg = np.zeros(n)
        np.add.at(deg, col2, ew2)
        dinv = np.where(deg > 0, deg**-0.5, 0.0)
        norm = dinv[row2] * ew2 * dinv[col2]
        out = np.zeros(n)
        np.add.at(out, col2, norm * xw_vec[row2])
        return out

    outs = []
    for c in range(3):
        ew = edge_attr[:, c]
        h1 = gcn(xw, ew) + g1b[0]
        h2 = gcn(h1 * g2W[0, 0], ew) + g2b[0]
        # SortPool: jnp.argsort(-h2) is a stable ascending sort of the negation
        perm = np.argsort(-h2, kind="stable")
        hs = np.stack([h1[perm], h2[perm]], axis=1)  # [n, 2]
        z = hs.T  # [2, n]
        L = z.shape[1] - 2
        z1 = np.zeros((3, L))
        for o in range(3):
            for i in range(2):
                for k in range(3):
                    z1[o] += c1w[o, i, k] * z[i, k : k + L]
            z1[o] += c1b[o]
        z1p = np.max(np.stack([z1[:, 0 : L - 2], z1[:, 1 : L - 1], z1[:, 2:L]], 0), 0)
        L2 = z1p.shape[1] - 2
        z2 = np.zeros((1, L2))
        for i in range(3):
            for k in range(3):
                z2[0] += c2w[0, i, k] * z1p[i, k : k + L2]
        z2[0] += c2b[0]
        z2p = np.max(
            np.stack([z2[:, 0 : L2 - 2], z2[:, 1 : L2 - 1], z2[:, 2:L2]], 0), 0
        )
        outs.append(z2p)  # [1, 121]

    allx = np.concatenate(outs, axis=0)  # [3, 121]
    h = allx.reshape(1, -1)

    def elu(v):
        return np.where(v > 0, v, np.expm1(v))

    h = elu(h @ f1W + f1b)
    h = elu(h @ f2W + f2b)
    out = h @ f3W + f3b
    return out.astype(np.float32)


def kernel(**inputs) -> np.ndarray:
    x = np.ascontiguousarray(np.asarray(inputs["x"], np.float32))
    w = np.asarray(inputs["gcn1_W"], np.float32).reshape(-1)
    xw = _matvec_device(x, w)
    return _downstream(xw, inputs)


# revision 8
# speedup vs baseline: 1.0307x; 1.0178x over previous
"""DGCNN (nn_DGCNN_43911745634410) Trainium2 kernel.

Structure of the model: the only heavy compute is xw = x @ gcn1_W with
x [129, 262144] f32 (~135 MB) and gcn1_W [262144, 1] — a memory-bound matvec.
xw is shared by all three edge-attr channels (it does not depend on edge
weights). Everything downstream (segment-sums over 16K edges, a 129-element
sort, two tiny conv1ds and three FCs) is a few hundred KFLOPs.

Device strategy (8 NeuronCores, tensor-parallel over the feature dim F):
  - core c gets x[:, c*32768:(c+1)*32768] (16.5 MB) and the matching w slice;
  - a raw-Bass kernel streams the shard through SBUF and uses the DVE's fused
    scalar_tensor_tensor (out=(x*1)*w, accum_out=free-dim sum) to produce
    per-partition partial dot products at one DVE pass per element, so the
    kernel runs at the HBM/DMA roofline (~47 us per core);
  - bulk tiles are [128, 1024] (4 rows x 32 partitions-per-row), the last row
    is one short [128, 256] tile so the non-overlapped tail op is short.
  - partials ([128, 33] per core) are summed on the host in f64 (all-reduce
    across cores), and the tiny downstream runs on the host in f64, exactly
    matching the reference semantics (stable descending sort, PyG GCN
    normalization with self-loops, VALID conv1d/maxpool, ELU MLP).

The raw-Bass (no TileContext) form is deliberate: this toolchain encodes at
most ONE semaphore wait per instruction, so each x tile gets a dedicated SBUF
buffer (the whole shard fits: ~132 KB/partition of the 224 KB) and every wait
is a single explicit wait_ge.
"""
from contextlib import ExitStack

import numpy as np

import concourse.bass as bass
from concourse import mybir
from concourse.bass_utils import run_bass_kernel_spmd

F32 = mybir.dt.float32

N = 129
F = 262144
NCORES = 8
SH = F // NCORES          # 32768 features per core
FD = 1024                 # free elems per partition per bulk tile
PPR = SH // FD            # partitions per row = 32
RPT = 128 // PPR          # rows per bulk tile = 4
NB = 27                   # bulk tiles [128, 1024], rows 0..107
TFD = SH // 128           # 256: small-tile free dim (one row per tile)
NS = 21                   # small tiles [128, 256], rows 108..128
NCOL = NB + NS            # 48 partial columns

_NC_CACHE = None


def _build_matvec_bass():
    nc = bass.Bass("TRN2")
    x = nc.dram_tensor("x_s", [N * SH], F32, kind="ExternalInput")
    w = nc.dram_tensor("w_s", [SH], F32, kind="ExternalInput")
    i32 = nc.dram_tensor("i32x4", [32, 128], F32, kind="ExternalInput")
    out = nc.dram_tensor("part", [128, NCOL], F32, kind="ExternalOutput")

    with ExitStack() as ctx:
        i32t = ctx.enter_context(nc.sbuf_tensor("i32t", [32, 128], F32))
        wq = ctx.enter_context(nc.sbuf_tensor("wq", [32, FD], F32))
        wnt = ctx.enter_context(nc.sbuf_tensor("wnt", [128, TFD], F32))
        wt_ps = ctx.enter_context(nc.psum_tensor("wt_ps", [128, FD], F32))
        wt_sb = ctx.enter_context(nc.sbuf_tensor("wt_sb", [128, FD], F32))
        xts = [
            ctx.enter_context(nc.sbuf_tensor(f"xt{t}", [128, FD], F32))
            for t in range(NB)
        ]
        xss = [
            ctx.enter_context(nc.sbuf_tensor(f"xs{s}", [128, TFD], F32))
            for s in range(NS)
        ]
        part = ctx.enter_context(nc.sbuf_tensor("part_sb", [128, NCOL], F32))
        w_sem = ctx.enter_context(nc.semaphore("w_sem"))
        pe_sem = ctx.enter_context(nc.semaphore("pe_sem"))
        act_sem = ctx.enter_context(nc.semaphore("act_sem"))
        wn_sem = ctx.enter_context(nc.semaphore("wn_sem"))
        x_sems = [ctx.enter_context(nc.semaphore(f"x_sem{t}")) for t in range(NB)]
        s_sems = [ctx.enter_context(nc.semaphore(f"s_sem{s}")) for s in range(NS)]
        dve_sem = ctx.enter_context(nc.semaphore("dve_sem"))
        out_sem = ctx.enter_context(nc.semaphore("out_sem"))
        block = ctx.enter_context(nc.Block())

        base = NB * 128 * FD

        @block.sync
        def _(sync):
            # x0 first: its 1.5us transfer hides the descriptor-gen of the
            # three tiny w/i32 loads (gen cadence ~650ns/DMA would otherwise
            # put ~1.1us of gaps at the stream head).
            src0 = x[0 : 128 * FD].rearrange("(p f) -> p f", f=FD)
            sync.dma_start(xts[0][:, :], src0).then_inc(x_sems[0], 16)
            sync.dma_start(i32t[:, :], i32[:, :]).then_inc(w_sem, 16)
            sync.dma_start(
                wq[:, :], w[:].rearrange("(q j) -> q j", j=FD)
            ).then_inc(w_sem, 16)
            for t in range(1, NB):
                src = x[t * 128 * FD : (t + 1) * 128 * FD].rearrange(
                    "(p f) -> p f", f=FD
                )
                sync.dma_start(xts[t][:, :], src).then_inc(x_sems[t], 16)
                if t == 1:
                    # wnat is only needed for the closing small tiles; its
                    # descriptor-gen hides under x1's transfer here
                    sync.dma_start(
                        wnt[:, :], w[:].rearrange("(p i) -> p i", i=TFD)
                    ).then_inc(wn_sem, 16)
            for s in range(NS):
                src = x[base + s * 128 * TFD : base + (s + 1) * 128 * TFD].rearrange(
                    "(p f) -> p f", f=TFD
                )
                sync.dma_start(xss[s][:, :], src).then_inc(s_sems[s], 16)
            sync.wait_ge(dve_sem, NCOL)
            sync.dma_start(out[:, :], part[:, :]).then_inc(out_sem, 16)

        @block.tensor
        def _(tensor):
            tensor.wait_ge(w_sem, 32)  # i32 + wq loaded
            nc.tensor.matmul(
                wt_ps[:, 0:512], i32t[:, :], wq[:, 0:512],
                start=True, stop=True,
            ).then_inc(pe_sem, 1)
            nc.tensor.matmul(
                wt_ps[:, 512:FD], i32t[:, :], wq[:, 512:FD],
                start=True, stop=True,
            ).then_inc(pe_sem, 1)

        @block.scalar
        def _(scalar):
            scalar.wait_ge(pe_sem, 2)
            nc.scalar.copy(wt_sb[:, :], wt_ps[:, :]).then_inc(act_sem, 1)

        @block.vector
        def _(vector):
            vector.wait_ge(act_sem, 1)
            for t in range(NB):
                vector.wait_ge(x_sems[t], 16)
                nc.vector.scalar_tensor_tensor(
                    xts[t][:, :],
                    xts[t][:, :],
                    1.0,
                    wt_sb[:, :],
                    op0=mybir.AluOpType.mult,
                    op1=mybir.AluOpType.mult,
                    accum_out=part[:, t : t + 1],
                ).then_inc(dve_sem, 1)
            vector.wait_ge(wn_sem, 16)
            for s in range(NS):
                vector.wait_ge(s_sems[s], 16)
                nc.vector.scalar_tensor_tensor(
                    xss[s][:, :],
                    xss[s][:, :],
                    1.0,
                    wnt[:, :],
                    op0=mybir.AluOpType.mult,
                    op1=mybir.AluOpType.mult,
                    accum_out=part[:, NB + s : NB + s + 1],
                ).then_inc(dve_sem, 1)

    return nc



def get_matvec_bass():
    global _NC_CACHE
    if _NC_CACHE is None:
        _NC_CACHE = _build_matvec_bass()
    return _NC_CACHE


def _make_core_inputs(x_np, w_np, core):
    xs = np.ascontiguousarray(x_np[:, core * SH : (core + 1) * SH]).reshape(-1)
    ws = np.ascontiguousarray(w_np[core * SH : (core + 1) * SH])
    i32x4 = np.ascontiguousarray(np.tile(np.eye(32, dtype=np.float32), (1, 4)))
    return {"x_s": xs, "w_s": ws, "i32x4": i32x4}


def _reduce_parts(parts):
    """parts: 8 arrays [128, NCOL] f32 -> xw [N] f64."""
    xw = np.zeros(N, np.float64)
    for part in parts:
        p = part.astype(np.float64)
        for t in range(NB):
            xw[RPT * t : RPT * (t + 1)] += p[:, t].reshape(RPT, PPR).sum(1)
        for si in range(NS):
            xw[RPT * NB + si] += p[:, NB + si].sum()
    return xw


def _matvec_device(x_np, w_np):
    """x [N, F] f32, w [F] f32 -> xw [N] f64 via the 8-core bass kernel."""
    global _NC_CACHE
    in_maps = [_make_core_inputs(x_np, w_np, c) for c in range(NCORES)]
    last_exc = None
    for attempt in range(2):
        try:
            nc = get_matvec_bass()
            res = run_bass_kernel_spmd(nc, in_maps, core_ids=list(range(NCORES)))
            return _reduce_parts([res.results[c]["part"] for c in range(NCORES)])
        except Exception as e:  # transient NRT_EXEC_UNIT_UNRECOVERABLE seen once
            import sys

            print(f"kernel: device run attempt {attempt} failed: {e!r:.200}",
                  file=sys.stderr)
            last_exc = e
            _NC_CACHE = None
    # Last-resort host fallback so a transient device failure still yields a
    # correct result (numerically equivalent partial-sum structure).
    import sys

    print(f"kernel: device path failed twice ({last_exc!r:.200}); "
          "falling back to host matvec", file=sys.stderr)
    prod = x_np.astype(np.float64) * w_np.astype(np.float64)[None, :]
    return prod.sum(axis=1)


def _downstream(xw, inputs):
    """Everything after xw = x @ gcn1_W, in f64 numpy. Returns [1, 2] f32."""
    edge_index = np.asarray(inputs["edge_index"]).astype(np.int64)
    row, col = edge_index[0], edge_index[1]
    edge_attr = np.asarray(inputs["edge_attr"], np.float64)
    g1b = np.asarray(inputs["gcn1_b"], np.float64)
    g2W = np.asarray(inputs["gcn2_W"], np.float64)
    g2b = np.asarray(inputs["gcn2_b"], np.float64)
    c1w = np.asarray(inputs["conv1_w"], np.float64)
    c1b = np.asarray(inputs["conv1_b"], np.float64)
    c2w = np.asarray(inputs["conv2_w"], np.float64)
    c2b = np.asarray(inputs["conv2_b"], np.float64)
    f1W = np.asarray(inputs["fc1_W"], np.float64)
    f1b = np.asarray(inputs["fc1_b"], np.float64)
    f2W = np.asarray(inputs["fc2_W"], np.float64)
    f2b = np.asarray(inputs["fc2_b"], np.float64)
    f3W = np.asarray(inputs["fc3_W"], np.float64)
    f3b = np.asarray(inputs["fc3_b"], np.float64)

    n = N
    loop = np.arange(n)
    row2 = np.concatenate([row, loop])
    col2 = np.concatenate([col, loop])

    def gcn(xw_vec, ew):
        # PyG GCNConv with edge weights: self-loops (weight 1), symmetric norm.
        ew2 = np.concatenate([ew, np.ones(n)])
        deg = np.zeros(n)
        np.add.at(deg, col2, ew2)
        dinv = np.where(deg > 0, deg**-0.5, 0.0)
        norm = dinv[row2] * ew2 * dinv[col2]
        out = np.zeros(n)
        np.add.at(out, col2, norm * xw_vec[row2])
        return out

    outs = []
    for c in range(3):
        ew = edge_attr[:, c]
        h1 = gcn(xw, ew) + g1b[0]
        h2 = gcn(h1 * g2W[0, 0], ew) + g2b[0]
        # SortPool: jnp.argsort(-h2) is a stable ascending sort of the negation
        perm = np.argsort(-h2, kind="stable")
        hs = np.stack([h1[perm], h2[perm]], axis=1)  # [n, 2]
        z = hs.T  # [2, n]
        L = z.shape[1] - 2
        z1 = np.zeros((3, L))
        for o in range(3):
            for i in range(2):
                for k in range(3):
                    z1[o] += c1w[o, i, k] * z[i, k : k + L]
            z1[o] += c1b[o]
        z1p = np.max(np.stack([z1[:, 0 : L - 2], z1[:, 1 : L - 1], z1[:, 2:L]], 0), 0)
        L2 = z1p.shape[1] - 2
        z2 = np.zeros((1, L2))
        for i in range(3):
            for k in range(3):
                z2[0] += c2w[0, i, k] * z1p[i, k : k + L2]
        z2[0] += c2b[0]
        z2p = np.max(
            np.stack([z2[:, 0 : L2 - 2], z2[:, 1 : L2 - 1], z2[:, 2:L2]], 0), 0
        )
        outs.append(z2p)  # [1, 121]

    allx = np.concatenate(outs, axis=0)  # [3, 121]
    h = allx.reshape(1, -1)

    def elu(v):
        return np.where(v > 0, v, np.expm1(v))

    h = elu(h @ f1W + f1b)
    h = elu(h @ f2W + f2b)
    out = h @ f3W + f3b
    return out.astype(np.float32)


def kernel(**inputs) -> np.ndarray:
    x = np.ascontiguousarray(np.asarray(inputs["x"], np.float32))
    w = np.asarray(inputs["gcn1_W"], np.float32).reshape(-1)
    xw = _matvec_device(x, w)
    return _downstream(xw, inputs)


# revision 9
# speedup vs baseline: 1.0368x; 1.0059x over previous
"""DGCNN (nn_DGCNN_43911745634410) Trainium2 kernel.

Structure of the model: the only heavy compute is xw = x @ gcn1_W with
x [129, 262144] f32 (~135 MB) and gcn1_W [262144, 1] — a memory-bound matvec.
xw is shared by all three edge-attr channels (it does not depend on edge
weights). Everything downstream (segment-sums over 16K edges, a 129-element
sort, two tiny conv1ds and three FCs) is a few hundred KFLOPs.

Device strategy (8 NeuronCores, tensor-parallel over the feature dim F):
  - core c gets x[:, c*32768:(c+1)*32768] (16.5 MB) and the matching w slice;
  - a raw-Bass kernel streams the shard through SBUF and uses the DVE's fused
    scalar_tensor_tensor (out=(x*1)*w, accum_out=free-dim sum) to produce
    per-partition partial dot products at one DVE pass per element, so the
    kernel runs at the HBM/DMA roofline (~47 us per core);
  - bulk tiles are [128, 1024] (4 rows x 32 partitions-per-row), the last row
    is one short [128, 256] tile so the non-overlapped tail op is short.
  - partials ([128, 33] per core) are summed on the host in f64 (all-reduce
    across cores), and the tiny downstream runs on the host in f64, exactly
    matching the reference semantics (stable descending sort, PyG GCN
    normalization with self-loops, VALID conv1d/maxpool, ELU MLP).

The raw-Bass (no TileContext) form is deliberate: this toolchain encodes at
most ONE semaphore wait per instruction, so each x tile gets a dedicated SBUF
buffer (the whole shard fits: ~132 KB/partition of the 224 KB) and every wait
is a single explicit wait_ge.
"""
from contextlib import ExitStack

import numpy as np

import concourse.bass as bass
from concourse import mybir
from concourse.bass_utils import run_bass_kernel_spmd

F32 = mybir.dt.float32

N = 129
F = 262144
NCORES = 8
SH = F // NCORES          # 32768 features per core
FD = 1024                 # free elems per partition per bulk tile
PPR = SH // FD            # partitions per row = 32
RPT = 128 // PPR          # rows per bulk tile = 4
NB = 27                   # bulk tiles [128, 1024], rows 0..107
TFD = SH // 128           # 256: small-tile free dim (one row per tile)
NS = 21                   # small tiles [128, 256], rows 108..128
NCOL = NB + NS            # 48 partial columns

_NC_CACHE = None


def _build_matvec_bass():
    nc = bass.Bass("TRN2")
    x = nc.dram_tensor("x_s", [N * SH], F32, kind="ExternalInput")
    w = nc.dram_tensor("w_s", [SH], F32, kind="ExternalInput")
    i32 = nc.dram_tensor("i32x4", [32, 128], F32, kind="ExternalInput")
    isel = nc.dram_tensor("isel", [32, 131], F32, kind="ExternalInput")
    out = nc.dram_tensor("part", [128, NCOL], F32, kind="ExternalOutput")

    with ExitStack() as ctx:
        i32t = ctx.enter_context(nc.sbuf_tensor("i32t", [32, 128], F32))
        iselt = ctx.enter_context(nc.sbuf_tensor("iselt", [32, 131], F32))
        wq = ctx.enter_context(nc.sbuf_tensor("wq", [32, FD], F32))
        wnt = ctx.enter_context(nc.sbuf_tensor("wnt", [128, TFD], F32))
        wt_ps = ctx.enter_context(nc.psum_tensor("wt_ps", [128, FD], F32))
        wn_ps = ctx.enter_context(nc.psum_tensor("wn_ps", [128, TFD], F32))
        wt_sb = ctx.enter_context(nc.sbuf_tensor("wt_sb", [128, FD], F32))
        xts = [
            ctx.enter_context(nc.sbuf_tensor(f"xt{t}", [128, FD], F32))
            for t in range(NB)
        ]
        xss = [
            ctx.enter_context(nc.sbuf_tensor(f"xs{s}", [128, TFD], F32))
            for s in range(NS)
        ]
        part = ctx.enter_context(nc.sbuf_tensor("part_sb", [128, NCOL], F32))
        w_sem = ctx.enter_context(nc.semaphore("w_sem"))
        pe_sem = ctx.enter_context(nc.semaphore("pe_sem"))
        act_sem = ctx.enter_context(nc.semaphore("act_sem"))
        isel_sem = ctx.enter_context(nc.semaphore("isel_sem"))
        x_sems = [ctx.enter_context(nc.semaphore(f"x_sem{t}")) for t in range(NB)]
        s_sems = [ctx.enter_context(nc.semaphore(f"s_sem{s}")) for s in range(NS)]
        dve_sem = ctx.enter_context(nc.semaphore("dve_sem"))
        out_sem = ctx.enter_context(nc.semaphore("out_sem"))
        block = ctx.enter_context(nc.Block())

        base = NB * 128 * FD

        @block.sync
        def _(sync):
            # x0 first: its 1.5us transfer hides the descriptor-gen of the
            # three tiny w/i32 loads (gen cadence ~650ns/DMA would otherwise
            # put ~1.1us of gaps at the stream head).
            src0 = x[0 : 128 * FD].rearrange("(p f) -> p f", f=FD)
            sync.dma_start(xts[0][:, :], src0).then_inc(x_sems[0], 16)
            sync.dma_start(i32t[:, :], i32[:, :]).then_inc(w_sem, 16)
            sync.dma_start(
                wq[:, :], w[:].rearrange("(q j) -> q j", j=FD)
            ).then_inc(w_sem, 16)
            for t in range(1, NB):
                src = x[t * 128 * FD : (t + 1) * 128 * FD].rearrange(
                    "(p f) -> p f", f=FD
                )
                sync.dma_start(xts[t][:, :], src).then_inc(x_sems[t], 16)
                if t == 1:
                    # isel gen hides under x1's transfer
                    sync.dma_start(iselt[:, :], isel[:, :]).then_inc(isel_sem, 16)
            for s in range(NS):
                src = x[base + s * 128 * TFD : base + (s + 1) * 128 * TFD].rearrange(
                    "(p f) -> p f", f=TFD
                )
                sync.dma_start(xss[s][:, :], src).then_inc(s_sems[s], 16)
            sync.wait_ge(dve_sem, NCOL)
            sync.dma_start(out[:, :], part[:, :]).then_inc(out_sem, 16)

        @block.tensor
        def _(tensor):
            tensor.wait_ge(w_sem, 32)  # i32 + wq loaded
            nc.tensor.matmul(
                wt_ps[:, 0:512], i32t[:, :], wq[:, 0:512],
                start=True, stop=True,
            ).then_inc(pe_sem, 1)
            nc.tensor.matmul(
                wt_ps[:, 512:FD], i32t[:, :], wq[:, 512:FD],
                start=True, stop=True,
            ).then_inc(pe_sem, 1)
            tensor.wait_ge(isel_sem, 16)
            # wn_ps[p, i] = wq[p//4, (p%4)*256 + i]: four accumulating
            # matmuls; lhsT_b = iselt[:, 3-b : 131-b] has ones at (q, 4q+b),
            # so pass b contributes rows p%4 == b and exact zeros elsewhere.
            for b in range(4):
                nc.tensor.matmul(
                    wn_ps[:, :], iselt[:, 3 - b : 131 - b],
                    wq[:, b * TFD : (b + 1) * TFD],
                    start=(b == 0), stop=(b == 3),
                ).then_inc(pe_sem, 1)

        @block.scalar
        def _(scalar):
            scalar.wait_ge(pe_sem, 2)
            nc.scalar.copy(wt_sb[:, :], wt_ps[:, :]).then_inc(act_sem, 1)
            scalar.wait_ge(pe_sem, 6)
            nc.scalar.copy(wnt[:, :], wn_ps[:, :]).then_inc(act_sem, 1)

        @block.vector
        def _(vector):
            vector.wait_ge(act_sem, 1)
            for t in range(NB):
                vector.wait_ge(x_sems[t], 16)
                nc.vector.scalar_tensor_tensor(
                    xts[t][:, :],
                    xts[t][:, :],
                    1.0,
                    wt_sb[:, :],
                    op0=mybir.AluOpType.mult,
                    op1=mybir.AluOpType.mult,
                    accum_out=part[:, t : t + 1],
                ).then_inc(dve_sem, 1)
            vector.wait_ge(act_sem, 2)
            for s in range(NS):
                vector.wait_ge(s_sems[s], 16)
                nc.vector.scalar_tensor_tensor(
                    xss[s][:, :],
                    xss[s][:, :],
                    1.0,
                    wnt[:, :],
                    op0=mybir.AluOpType.mult,
                    op1=mybir.AluOpType.mult,
                    accum_out=part[:, NB + s : NB + s + 1],
                ).then_inc(dve_sem, 1)

    return nc



def get_matvec_bass():
    global _NC_CACHE
    if _NC_CACHE is None:
        _NC_CACHE = _build_matvec_bass()
    return _NC_CACHE


def _make_core_inputs(x_np, w_np, core):
    xs = np.ascontiguousarray(x_np[:, core * SH : (core + 1) * SH]).reshape(-1)
    ws = np.ascontiguousarray(w_np[core * SH : (core + 1) * SH])
    i32x4 = np.ascontiguousarray(np.tile(np.eye(32, dtype=np.float32), (1, 4)))
    isel = np.zeros((32, 131), np.float32)
    isel[np.arange(32), 3 + 4 * np.arange(32)] = 1.0
    return {"x_s": xs, "w_s": ws, "i32x4": i32x4, "isel": isel}


def _reduce_parts(parts):
    """parts: 8 arrays [128, NCOL] f32 -> xw [N] f64."""
    xw = np.zeros(N, np.float64)
    for part in parts:
        p = part.astype(np.float64)
        for t in range(NB):
            xw[RPT * t : RPT * (t + 1)] += p[:, t].reshape(RPT, PPR).sum(1)
        for si in range(NS):
            xw[RPT * NB + si] += p[:, NB + si].sum()
    return xw


def _matvec_device(x_np, w_np):
    """x [N, F] f32, w [F] f32 -> xw [N] f64 via the 8-core bass kernel."""
    global _NC_CACHE
    in_maps = [_make_core_inputs(x_np, w_np, c) for c in range(NCORES)]
    last_exc = None
    for attempt in range(2):
        try:
            nc = get_matvec_bass()
            res = run_bass_kernel_spmd(nc, in_maps, core_ids=list(range(NCORES)))
            return _reduce_parts([res.results[c]["part"] for c in range(NCORES)])
        except Exception as e:  # transient NRT_EXEC_UNIT_UNRECOVERABLE seen once
            import sys

            print(f"kernel: device run attempt {attempt} failed: {e!r:.200}",
                  file=sys.stderr)
            last_exc = e
            _NC_CACHE = None
    # Last-resort host fallback so a transient device failure still yields a
    # correct result (numerically equivalent partial-sum structure).
    import sys

    print(f"kernel: device path failed twice ({last_exc!r:.200}); "
          "falling back to host matvec", file=sys.stderr)
    prod = x_np.astype(np.float64) * w_np.astype(np.float64)[None, :]
    return prod.sum(axis=1)


def _downstream(xw, inputs):
    """Everything after xw = x @ gcn1_W, in f64 numpy. Returns [1, 2] f32."""
    edge_index = np.asarray(inputs["edge_index"]).astype(np.int64)
    row, col = edge_index[0], edge_index[1]
    edge_attr = np.asarray(inputs["edge_attr"], np.float64)
    g1b = np.asarray(inputs["gcn1_b"], np.float64)
    g2W = np.asarray(inputs["gcn2_W"], np.float64)
    g2b = np.asarray(inputs["gcn2_b"], np.float64)
    c1w = np.asarray(inputs["conv1_w"], np.float64)
    c1b = np.asarray(inputs["conv1_b"], np.float64)
    c2w = np.asarray(inputs["conv2_w"], np.float64)
    c2b = np.asarray(inputs["conv2_b"], np.float64)
    f1W = np.asarray(inputs["fc1_W"], np.float64)
    f1b = np.asarray(inputs["fc1_b"], np.float64)
    f2W = np.asarray(inputs["fc2_W"], np.float64)
    f2b = np.asarray(inputs["fc2_b"], np.float64)
    f3W = np.asarray(inputs["fc3_W"], np.float64)
    f3b = np.asarray(inputs["fc3_b"], np.float64)

    n = N
    loop = np.arange(n)
    row2 = np.concatenate([row, loop])
    col2 = np.concatenate([col, loop])

    def gcn(xw_vec, ew):
        # PyG GCNConv with edge weights: self-loops (weight 1), symmetric norm.
        ew2 = np.concatenate([ew, np.ones(n)])
        deg = np.zeros(n)
        np.add.at(deg, col2, ew2)
        dinv = np.where(deg > 0, deg**-0.5, 0.0)
        norm = dinv[row2] * ew2 * dinv[col2]
        out = np.zeros(n)
        np.add.at(out, col2, norm * xw_vec[row2])
        return out

    outs = []
    for c in range(3):
        ew = edge_attr[:, c]
        h1 = gcn(xw, ew) + g1b[0]
        h2 = gcn(h1 * g2W[0, 0], ew) + g2b[0]
        # SortPool: jnp.argsort(-h2) is a stable ascending sort of the negation
        perm = np.argsort(-h2, kind="stable")
        hs = np.stack([h1[perm], h2[perm]], axis=1)  # [n, 2]
        z = hs.T  # [2, n]
        L = z.shape[1] - 2
        z1 = np.zeros((3, L))
        for o in range(3):
            for i in range(2):
                for k in range(3):
                    z1[o] += c1w[o, i, k] * z[i, k : k + L]
            z1[o] += c1b[o]
        z1p = np.max(np.stack([z1[:, 0 : L - 2], z1[:, 1 : L - 1], z1[:, 2:L]], 0), 0)
        L2 = z1p.shape[1] - 2
        z2 = np.zeros((1, L2))
        for i in range(3):
            for k in range(3):
                z2[0] += c2w[0, i, k] * z1p[i, k : k + L2]
        z2[0] += c2b[0]
        z2p = np.max(
            np.stack([z2[:, 0 : L2 - 2], z2[:, 1 : L2 - 1], z2[:, 2:L2]], 0), 0
        )
        outs.append(z2p)  # [1, 121]

    allx = np.concatenate(outs, axis=0)  # [3, 121]
    h = allx.reshape(1, -1)

    def elu(v):
        return np.where(v > 0, v, np.expm1(v))

    h = elu(h @ f1W + f1b)
    h = elu(h @ f2W + f2b)
    out = h @ f3W + f3b
    return out.astype(np.float32)


def kernel(**inputs) -> np.ndarray:
    x = np.ascontiguousarray(np.asarray(inputs["x"], np.float32))
    w = np.asarray(inputs["gcn1_W"], np.float32).reshape(-1)
    xw = _matvec_device(x, w)
    return _downstream(xw, inputs)


# revision 10
# speedup vs baseline: 1.0378x; 1.0009x over previous
"""DGCNN (nn_DGCNN_43911745634410) Trainium2 kernel.

Structure of the model: the only heavy compute is xw = x @ gcn1_W with
x [129, 262144] f32 (~135 MB) and gcn1_W [262144, 1] — a memory-bound matvec.
xw is shared by all three edge-attr channels (it does not depend on edge
weights). Everything downstream (segment-sums over 16K edges, a 129-element
sort, two tiny conv1ds and three FCs) is a few hundred KFLOPs.

Device strategy (8 NeuronCores, tensor-parallel over the feature dim F):
  - core c gets x[:, c*32768:(c+1)*32768] (16.5 MB) and the matching w slice;
  - a raw-Bass kernel streams the shard through SBUF and uses the DVE's fused
    scalar_tensor_tensor (out=(x*1)*w, accum_out=free-dim sum) to produce
    per-partition partial dot products at one DVE pass per element, so the
    kernel runs at the HBM/DMA roofline (~47 us per core);
  - bulk tiles are [128, 1024] (4 rows x 32 partitions-per-row), the last row
    is one short [128, 256] tile so the non-overlapped tail op is short.
  - partials ([128, 33] per core) are summed on the host in f64 (all-reduce
    across cores), and the tiny downstream runs on the host in f64, exactly
    matching the reference semantics (stable descending sort, PyG GCN
    normalization with self-loops, VALID conv1d/maxpool, ELU MLP).

The raw-Bass (no TileContext) form is deliberate: this toolchain encodes at
most ONE semaphore wait per instruction, so each x tile gets a dedicated SBUF
buffer (the whole shard fits: ~132 KB/partition of the 224 KB) and every wait
is a single explicit wait_ge.
"""
from contextlib import ExitStack

import numpy as np

import concourse.bass as bass
from concourse import mybir
from concourse.bass_utils import run_bass_kernel_spmd

F32 = mybir.dt.float32

N = 129
F = 262144
NCORES = 8
SH = F // NCORES          # 32768 features per core
FD = 1024                 # free elems per partition per bulk tile
PPR = SH // FD            # partitions per row = 32
RPT = 128 // PPR          # rows per bulk tile = 4
NB = 27                   # bulk tiles [128, 1024], rows 0..107
TFD = SH // 128           # 256: small-tile free dim (one row per tile)
NS = 21                   # small tiles [128, 256], rows 108..128
NCOL = NB + NS            # 48 partial columns

_NC_CACHE = None


def _build_matvec_bass():
    nc = bass.Bass("TRN2")
    x = nc.dram_tensor("x_s", [N * SH], F32, kind="ExternalInput")
    w = nc.dram_tensor("w_s", [SH], F32, kind="ExternalInput")
    sel = nc.dram_tensor("sel", [32, 259], F32, kind="ExternalInput")
    out = nc.dram_tensor("part", [128, NCOL], F32, kind="ExternalOutput")

    with ExitStack() as ctx:
        selt = ctx.enter_context(nc.sbuf_tensor("selt", [32, 259], F32))
        wq = ctx.enter_context(nc.sbuf_tensor("wq", [32, FD], F32))
        wnt = ctx.enter_context(nc.sbuf_tensor("wnt", [128, TFD], F32))
        wt_ps = ctx.enter_context(nc.psum_tensor("wt_ps", [128, FD], F32))
        wn_ps = ctx.enter_context(nc.psum_tensor("wn_ps", [128, TFD], F32))
        wt_sb = ctx.enter_context(nc.sbuf_tensor("wt_sb", [128, FD], F32))
        xts = [
            ctx.enter_context(nc.sbuf_tensor(f"xt{t}", [128, FD], F32))
            for t in range(NB)
        ]
        xss = [
            ctx.enter_context(nc.sbuf_tensor(f"xs{s}", [128, TFD], F32))
            for s in range(NS)
        ]
        part = ctx.enter_context(nc.sbuf_tensor("part_sb", [128, NCOL], F32))
        w_sem = ctx.enter_context(nc.semaphore("w_sem"))
        pe_sem = ctx.enter_context(nc.semaphore("pe_sem"))
        act_sem = ctx.enter_context(nc.semaphore("act_sem"))
        x_sems = [ctx.enter_context(nc.semaphore(f"x_sem{t}")) for t in range(NB)]
        s_sems = [ctx.enter_context(nc.semaphore(f"s_sem{s}")) for s in range(NS)]
        dve_sem = ctx.enter_context(nc.semaphore("dve_sem"))
        out_sem = ctx.enter_context(nc.semaphore("out_sem"))
        block = ctx.enter_context(nc.Block())

        base = NB * 128 * FD

        @block.sync
        def _(sync):
            # x0 first: its 1.5us transfer hides the descriptor-gen of the
            # three tiny w/i32 loads (gen cadence ~650ns/DMA would otherwise
            # put ~1.1us of gaps at the stream head).
            src0 = x[0 : 128 * FD].rearrange("(p f) -> p f", f=FD)
            sync.dma_start(xts[0][:, :], src0).then_inc(x_sems[0], 16)
            sync.dma_start(selt[:, :], sel[:, :]).then_inc(w_sem, 16)
            sync.dma_start(
                wq[:, :], w[:].rearrange("(q j) -> q j", j=FD)
            ).then_inc(w_sem, 16)
            for t in range(1, NB):
                src = x[t * 128 * FD : (t + 1) * 128 * FD].rearrange(
                    "(p f) -> p f", f=FD
                )
                sync.dma_start(xts[t][:, :], src).then_inc(x_sems[t], 16)
            for s in range(NS):
                src = x[base + s * 128 * TFD : base + (s + 1) * 128 * TFD].rearrange(
                    "(p f) -> p f", f=TFD
                )
                sync.dma_start(xss[s][:, :], src).then_inc(s_sems[s], 16)
            sync.wait_ge(dve_sem, NCOL)
            sync.dma_start(out[:, :], part[:, :]).then_inc(out_sem, 16)

        @block.tensor
        def _(tensor):
            tensor.wait_ge(w_sem, 32)  # sel + wq loaded
            nc.tensor.matmul(
                wt_ps[:, 0:512], selt[:, 0:128], wq[:, 0:512],
                start=True, stop=True,
            ).then_inc(pe_sem, 1)
            nc.tensor.matmul(
                wt_ps[:, 512:FD], selt[:, 0:128], wq[:, 512:FD],
                start=True, stop=True,
            ).then_inc(pe_sem, 1)
            # wn_ps[p, i] = wq[p//4, (p%4)*256 + i]: four accumulating
            # matmuls; lhsT_b = iselt[:, 3-b : 131-b] has ones at (q, 4q+b),
            # so pass b contributes rows p%4 == b and exact zeros elsewhere.
            for b in range(4):
                nc.tensor.matmul(
                    wn_ps[:, :], selt[:, 131 - b : 259 - b],
                    wq[:, b * TFD : (b + 1) * TFD],
                    start=(b == 0), stop=(b == 3),
                ).then_inc(pe_sem, 1)

        @block.scalar
        def _(scalar):
            scalar.wait_ge(pe_sem, 2)
            nc.scalar.copy(wt_sb[:, :], wt_ps[:, :]).then_inc(act_sem, 1)
            scalar.wait_ge(pe_sem, 6)
            nc.scalar.copy(wnt[:, :], wn_ps[:, :]).then_inc(act_sem, 1)

        @block.vector
        def _(vector):
            vector.wait_ge(act_sem, 1)
            for t in range(NB):
                vector.wait_ge(x_sems[t], 16)
                nc.vector.scalar_tensor_tensor(
                    xts[t][:, :],
                    xts[t][:, :],
                    1.0,
                    wt_sb[:, :],
                    op0=mybir.AluOpType.mult,
                    op1=mybir.AluOpType.mult,
                    accum_out=part[:, t : t + 1],
                ).then_inc(dve_sem, 1)
            vector.wait_ge(act_sem, 2)
            for s in range(NS):
                vector.wait_ge(s_sems[s], 16)
                nc.vector.scalar_tensor_tensor(
                    xss[s][:, :],
                    xss[s][:, :],
                    1.0,
                    wnt[:, :],
                    op0=mybir.AluOpType.mult,
                    op1=mybir.AluOpType.mult,
                    accum_out=part[:, NB + s : NB + s + 1],
                ).then_inc(dve_sem, 1)

    return nc



def get_matvec_bass():
    global _NC_CACHE
    if _NC_CACHE is None:
        _NC_CACHE = _build_matvec_bass()
    return _NC_CACHE


def _make_core_inputs(x_np, w_np, core):
    xs = np.ascontiguousarray(x_np[:, core * SH : (core + 1) * SH]).reshape(-1)
    ws = np.ascontiguousarray(w_np[core * SH : (core + 1) * SH])
    sel = np.zeros((32, 259), np.float32)
    sel[:, 0:128] = np.tile(np.eye(32, dtype=np.float32), (1, 4))
    sel[np.arange(32), 131 + 4 * np.arange(32)] = 1.0
    return {"x_s": xs, "w_s": ws, "sel": sel}


def _reduce_parts(parts):
    """parts: 8 arrays [128, NCOL] f32 -> xw [N] f64."""
    xw = np.zeros(N, np.float64)
    for part in parts:
        p = part.astype(np.float64)
        for t in range(NB):
            xw[RPT * t : RPT * (t + 1)] += p[:, t].reshape(RPT, PPR).sum(1)
        for si in range(NS):
            xw[RPT * NB + si] += p[:, NB + si].sum()
    return xw


def _matvec_device(x_np, w_np):
    """x [N, F] f32, w [F] f32 -> xw [N] f64 via the 8-core bass kernel."""
    global _NC_CACHE
    in_maps = [_make_core_inputs(x_np, w_np, c) for c in range(NCORES)]
    last_exc = None
    for attempt in range(2):
        try:
            nc = get_matvec_bass()
            res = run_bass_kernel_spmd(nc, in_maps, core_ids=list(range(NCORES)))
            return _reduce_parts([res.results[c]["part"] for c in range(NCORES)])
        except Exception as e:  # transient NRT_EXEC_UNIT_UNRECOVERABLE seen once
            import sys

            print(f"kernel: device run attempt {attempt} failed: {e!r:.200}",
                  file=sys.stderr)
            last_exc = e
            _NC_CACHE = None
    # Last-resort host fallback so a transient device failure still yields a
    # correct result (numerically equivalent partial-sum structure).
    import sys

    print(f"kernel: device path failed twice ({last_exc!r:.200}); "
          "falling back to host matvec", file=sys.stderr)
    prod = x_np.astype(np.float64) * w_np.astype(np.float64)[None, :]
    return prod.sum(axis=1)


def _downstream(xw, inputs):
    """Everything after xw = x @ gcn1_W, in f64 numpy. Returns [1, 2] f32."""
    edge_index = np.asarray(inputs["edge_index"]).astype(np.int64)
    row, col = edge_index[0], edge_index[1]
    edge_attr = np.asarray(inputs["edge_attr"], np.float64)
    g1b = np.asarray(inputs["gcn1_b"], np.float64)
    g2W = np.asarray(inputs["gcn2_W"], np.float64)
    g2b = np.asarray(inputs["gcn2_b"], np.float64)
    c1w = np.asarray(inputs["conv1_w"], np.float64)
    c1b = np.asarray(inputs["conv1_b"], np.float64)
    c2w = np.asarray(inputs["conv2_w"], np.float64)
    c2b = np.asarray(inputs["conv2_b"], np.float64)
    f1W = np.asarray(inputs["fc1_W"], np.float64)
    f1b = np.asarray(inputs["fc1_b"], np.float64)
    f2W = np.asarray(inputs["fc2_W"], np.float64)
    f2b = np.asarray(inputs["fc2_b"], np.float64)
    f3W = np.asarray(inputs["fc3_W"], np.float64)
    f3b = np.asarray(inputs["fc3_b"], np.float64)

    n = N
    loop = np.arange(n)
    row2 = np.concatenate([row, loop])
    col2 = np.concatenate([col, loop])

    def gcn(xw_vec, ew):
        # PyG GCNConv with edge weights: self-loops (weight 1), symmetric norm.
        ew2 = np.concatenate([ew, np.ones(n)])
        deg = np.zeros(n)
        np.add.at(deg, col2, ew2)
        dinv = np.where(deg > 0, deg**-0.5, 0.0)
        norm = dinv[row2] * ew2 * dinv[col2]
        out = np.zeros(n)
        np.add.at(out, col2, norm * xw_vec[row2])
        return out

    outs = []
    for c in range(3):
        ew = edge_attr[:, c]
        h1 = gcn(xw, ew) + g1b[0]
        h2 = gcn(h1 * g2W[0, 0], ew) + g2b[0]
        # SortPool: jnp.argsort(-h2) is a stable ascending sort of the negation
        perm = np.argsort(-h2, kind="stable")
        hs = np.stack([h1[perm], h2[perm]], axis=1)  # [n, 2]
        z = hs.T  # [2, n]
        L = z.shape[1] - 2
        z1 = np.zeros((3, L))
        for o in range(3):
            for i in range(2):
                for k in range(3):
                    z1[o] += c1w[o, i, k] * z[i, k : k + L]
            z1[o] += c1b[o]
        z1p = np.max(np.stack([z1[:, 0 : L - 2], z1[:, 1 : L - 1], z1[:, 2:L]], 0), 0)
        L2 = z1p.shape[1] - 2
        z2 = np.zeros((1, L2))
        for i in range(3):
            for k in range(3):
                z2[0] += c2w[0, i, k] * z1p[i, k : k + L2]
        z2[0] += c2b[0]
        z2p = np.max(
            np.stack([z2[:, 0 : L2 - 2], z2[:, 1 : L2 - 1], z2[:, 2:L2]], 0), 0
        )
        outs.append(z2p)  # [1, 121]

    allx = np.concatenate(outs, axis=0)  # [3, 121]
    h = allx.reshape(1, -1)

    def elu(v):
        return np.where(v > 0, v, np.expm1(v))

    h = elu(h @ f1W + f1b)
    h = elu(h @ f2W + f2b)
    out = h @ f3W + f3b
    return out.astype(np.float32)


def kernel(**inputs) -> np.ndarray:
    x = np.ascontiguousarray(np.asarray(inputs["x"], np.float32))
    w = np.asarray(inputs["gcn1_W"], np.float32).reshape(-1)
    xw = _matvec_device(x, w)
    return _downstream(xw, inputs)
